# revision 1
# baseline (speedup 1.0000x reference)
"""Trainium2 Bass kernel for CompressedLinear: out = x @ (w_int8 * scale).T + bias.

Sharding (Megatron column-parallel): weight/scale/bias are split along the
output dim across 8 NeuronCores, x is replicated, per-core outputs are
concatenated on the feature axis.

Key identity: x @ (w*scale).T + bias == (x @ w.T) * scale + bias, so the
matmul runs on the raw int8 codes (exactly representable in fp16/bf16) and
the per-output-channel scale/bias are applied on PSUM eviction by the vector
engine.  Precision: default is one fp16 pass (codes exact, x rounded to 11
mantissa bits, fp32 PSUM accumulation) -> ~2.5e-4 relative error at full PE
rate.  N_PASSES=2 splits x host-side into x_hi + x_lo for ~3e-6 at twice the
matmul cost; X_DT="bf16" is the 8-mantissa-bit variant (~2e-3).
"""

import numpy as np
import ml_dtypes

import concourse.bass as bass
import concourse.mybir as mybir
import concourse.tile as tile
from concourse import bacc
from concourse.bass_utils import run_bass_kernel_spmd

B, S, IN, OUT = 4, 2048, 4096, 11008
N_CORES = 8
TOK = B * S
O_CORE = OUT // N_CORES
P = 128

N_PASSES = 1
M_TILE = 256
N_TILE = 512
X_BUFS = 3
PSUM_BUFS = 4
X_DT = "fp16"  # "fp16" (11 mantissa bits) or "bf16" (8) — both full PE rate

_MYBIR_DT = {"fp16": mybir.dt.float16, "bf16": mybir.dt.bfloat16}
_NP_DT = {"fp16": np.float16, "bf16": ml_dtypes.bfloat16}


def build_nc(tok=TOK, k_dim=IN, o_core=O_CORE, n_passes=N_PASSES,
             m_tile=M_TILE, n_tile=N_TILE, x_bufs=X_BUFS, psum_bufs=PSUM_BUFS,
             x_dt=X_DT):
    assert k_dim % P == 0 and tok % m_tile == 0 and m_tile % P == 0
    ksub = k_dim // P
    msub = m_tile // P
    n_slices = [(n0, min(n_tile, o_core - n0)) for n0 in range(0, o_core, n_tile)]
    mm_dt = _MYBIR_DT[x_dt]

    nc = bacc.Bacc(None, target_bir_lowering=False)
    xt_names = ["xt_hi", "xt_lo"][:n_passes]
    xts = [nc.declare_dram_parameter(nm, [k_dim, tok], mm_dt, False)
           for nm in xt_names]
    wt = nc.declare_dram_parameter("wt", [k_dim, o_core], mm_dt, False)
    scale = nc.declare_dram_parameter("scale", [o_core], mybir.dt.float32, False)
    bias = nc.declare_dram_parameter("bias", [o_core], mybir.dt.float32, False)
    out = nc.declare_dram_parameter("out", [tok, o_core], mybir.dt.float32, True)

    with tile.TileContext(nc) as tc:
        with (
            tc.tile_pool(name="const", bufs=1) as cpool,
            tc.tile_pool(name="xp", bufs=x_bufs) as xpool,
            tc.tile_pool(name="op", bufs=2) as opool,
            tc.tile_pool(name="ps", bufs=psum_bufs, space="PSUM") as pspool,
        ):
            xt_res = [x.rearrange("(ks p) t -> p ks t", p=P) for x in xts]
            out_re = out.rearrange("(m p) o -> m p o", p=P)
            wt_re = wt.rearrange("(ks p) o -> p ks o", p=P)

            # Startup queue order on the sync ring: m-tile 0's x first, then
            # the resident weights chunked per (n-slice, k-group) in first-
            # consumption order, so the first matmuls start after ~3MB of
            # DMA instead of the full 13MB of startup traffic.
            x_pre = []
            for xi, xre in enumerate(xt_res):
                x_sb = xpool.tile([P, ksub, m_tile], mm_dt, tag=f"x{xi}")
                nc.sync.dma_start(out=x_sb[:], in_=xre[:, :, 0:m_tile])
                x_pre.append(x_sb)

            KG = 8
            w_tiles = {}
            for n_idx, (n0, nsz) in enumerate(n_slices):
                for kg in range(0, ksub, KG):
                    kgn = min(KG, ksub - kg)
                    w_t = cpool.tile([P, kgn, nsz], mm_dt, tag=f"w_{n_idx}_{kg}")
                    nc.sync.dma_start(out=w_t[:],
                                      in_=wt_re[:, kg:kg + kgn, n0:n0 + nsz])
                    w_tiles[(n_idx, kg // KG)] = w_t
            scale_sb = cpool.tile([P, o_core], mybir.dt.float32)
            nc.gpsimd.dma_start(out=scale_sb[:],
                                in_=scale[None, :].to_broadcast((P, o_core)))
            bias_sb = cpool.tile([P, o_core], mybir.dt.float32)
            nc.gpsimd.dma_start(out=bias_sb[:],
                                in_=bias[None, :].to_broadcast((P, o_core)))

            n_mm = n_passes * ksub
            for mi in range(tok // m_tile):
                if mi == 0:
                    x_sbs = x_pre
                else:
                    x_sbs = []
                    for xi, xre in enumerate(xt_res):
                        x_sb = xpool.tile([P, ksub, m_tile], mm_dt, tag=f"x{xi}")
                        nc.sync.dma_start(out=x_sb[:],
                                          in_=xre[:, :, mi * m_tile:(mi + 1) * m_tile])
                        x_sbs.append(x_sb)
                out_sb = opool.tile([P, msub, o_core], mybir.dt.float32, tag="o")
                for ms in range(msub):
                    for n_idx, (n0, nsz) in enumerate(n_slices):
                        ps = pspool.tile([P, n_tile], mybir.dt.float32, tag="ps")
                        psv = ps[:, :nsz]
                        i_mm = 0
                        for x_sb in x_sbs:
                            for ks in range(ksub):
                                nc.tensor.matmul(
                                    psv,
                                    lhsT=x_sb[:, ks, ms * P:(ms + 1) * P],
                                    rhs=w_tiles[(n_idx, ks // KG)][:, ks % KG, :],
                                    start=(i_mm == 0),
                                    stop=(i_mm == n_mm - 1),
                                )
                                i_mm += 1
                        nc.vector.tensor_mul(out=out_sb[:, ms, n0:n0 + nsz],
                                             in0=psv, in1=scale_sb[:, n0:n0 + nsz])
                        nc.vector.tensor_add(out=out_sb[:, ms, n0:n0 + nsz],
                                             in0=out_sb[:, ms, n0:n0 + nsz],
                                             in1=bias_sb[:, n0:n0 + nsz])
                        nc.sync.dma_start(
                            out=out_re[mi * msub + ms][:, n0:n0 + nsz],
                            in_=out_sb[:, ms, n0:n0 + nsz])
    nc.compile()
    return nc


def _shard_inputs(x2d, w, scale, bias, n_passes, n_cores, o_core, x_dt=X_DT):
    np_dt = _NP_DT[x_dt]
    xt32 = np.ascontiguousarray(x2d.T)  # [IN, TOK] f32
    xt_hi = xt32.astype(np_dt)
    common = {"xt_hi": xt_hi}
    if n_passes == 2:
        common["xt_lo"] = (xt32 - xt_hi.astype(np.float32)).astype(np_dt)
    # int codes in [0, 127) are exactly representable in bf16 and fp16
    wt = np.ascontiguousarray(w.T).astype(np_dt)  # [IN, OUT]
    in_maps = []
    for c in range(n_cores):
        sl = slice(c * o_core, (c + 1) * o_core)
        in_maps.append({
            **common,
            "wt": np.ascontiguousarray(wt[:, sl]),
            "scale": np.ascontiguousarray(scale[sl]),
            "bias": np.ascontiguousarray(bias[sl]),
        })
    return in_maps


def _ensure_ntff_hook():
    """Register the axon NTFF profiling hook if the image's antenv lacks it."""
    import sys, types
    try:
        from antenv.axon_hooks import get_axon_ntff_profile_hook  # noqa: F401
        return
    except ImportError:
        pass
    try:
        import antenv
        from trn_agent_boot.trn_boot import _ntff_profile_via_ctypes
        mod = types.ModuleType("antenv.axon_hooks")
        _hook = [_ntff_profile_via_ctypes("/opt/axon/libaxon_pjrt.so")]
        mod.set_axon_ntff_profile_hook = lambda h: _hook.__setitem__(0, h)
        mod.get_axon_ntff_profile_hook = lambda: _hook[0]
        sys.modules["antenv.axon_hooks"] = mod
        antenv.axon_hooks = mod
    except Exception as e:  # profiling is best-effort; execution still works
        print(f"NTFF hook registration failed: {e}")


def run_hw(x2d, w, scale, bias, trace=False, **build_kwargs):
    """Run sharded on 8 cores; returns (full [TOK, OUT] f32 output, exec_time_ns)."""
    if trace:
        _ensure_ntff_hook()
    nc = build_nc(**build_kwargs)
    in_maps = _shard_inputs(x2d, w, scale, bias,
                            build_kwargs.get("n_passes", N_PASSES), N_CORES, O_CORE,
                            build_kwargs.get("x_dt", X_DT))
    last_err = None
    for attempt in range(3):
        try:
            res = run_bass_kernel_spmd(nc, in_maps, core_ids=list(range(N_CORES)),
                                       trace=trace)
            out = np.concatenate([res.results[c]["out"] for c in range(N_CORES)],
                                 axis=1)
            return out, res.exec_time_ns
        except Exception as e:  # transient NRT_EXEC_UNIT_UNRECOVERABLE etc.
            last_err = e
            print(f"run attempt {attempt} failed: {type(e).__name__}: {e}")
            try:
                import jax
                import jax.extend.backend as _jb
                jax.clear_caches()
                _jb.clear_backends()
            except Exception as e2:
                print(f"backend reset failed: {e2}")
            import time
            time.sleep(5)
    raise last_err


def kernel(**inputs):
    x = np.asarray(inputs["x"], dtype=np.float32)
    w = np.asarray(inputs["weight_int8"])
    scale = np.asarray(inputs["scale"], dtype=np.float32)
    bias = np.asarray(inputs["bias"], dtype=np.float32)
    out2d, _ = run_hw(x.reshape(TOK, IN), w, scale, bias, trace=False)
    return out2d.reshape(B, S, OUT)



# revision 2
# speedup vs baseline: 1.4219x; 1.4219x over previous
"""Trainium2 Bass kernel for CompressedLinear: out = x @ (w_int8 * scale).T + bias.

Sharding (Megatron column-parallel): weight/scale/bias are split along the
output dim across 8 NeuronCores, x is replicated, per-core outputs are
concatenated on the feature axis.

Strategy: fp8 e4m3 matmul in DoubleRow perf mode (2 K-slices contracted per
instruction -> 2x the fp16 PE rate).  Numerics to stay under the 2e-2 gate:
  - w codes [0,126] are mean-shifted (w-63) and scaled by ALPHA=1.0125 before
    the fp8 cast (the scan-optimized grid alignment: E[err^2] 0.88 -> 0.69);
    ALPHA and the shift are undone exactly at eviction (scale_eff = scale/ALPHA
    and a rank-1 token correction c_t = ALPHA*63*rowsum(x) added to PSUM,
    rowsums computed on host in f64).
  - x is cast to fp8; for the first N_EXTRA*128 of the 4096 K-dims the fp8
    residual (x - fp8(x)) is appended as extra K-slices against duplicated w
    slices, removing that fraction of the x-side quantization error.
    N_EXTRA=10 -> simulated rel err 1.892e-2 at 1.3125x the fp8 GEMM cost.

Layout: output channels live on PSUM partitions (128 per n-group, 11 groups =
1408 padded channels per core), tokens on the free dim.  scale/bias are then
per-partition operands of one DVE tensor_scalar, and the token correction is a
free-dim broadcast tile added with one DVE tensor_add.  Output is stored
channel-major [1408, TOK] per core (contiguous 1KB DMA lines) and transposed
on the host after the gather.
"""

import numpy as np
import ml_dtypes

import concourse.bass as bass
import concourse.mybir as mybir
import concourse.tile as tile
from concourse import bacc
from concourse.bass_utils import run_bass_kernel_spmd

B, S, IN, OUT = 4, 2048, 4096, 11008
N_CORES = 8
TOK = B * S
O_CORE = OUT // N_CORES          # 1376
P = 128
NG = 11                          # n-groups of 128 channels (1408 padded)
O_PAD = NG * P

N_EXTRA = 10                     # extra K-slices carrying x fp8 residuals
ALPHA = 1.0125                   # w quantizer grid scale (undone via scale_eff)
SHIFT = 63.0                     # w mean shift (undone via the rank-1 c term)
M_TILE = 512                     # tokens per x DMA tile
M_SUB = 256                      # tokens per PSUM tile (moving free = 512)
X_BUFS = 3
PSUM_BUFS = 6

FP8 = ml_dtypes.float8_e4m3


def build_nc(n_extra=N_EXTRA, m_tile=M_TILE, x_bufs=X_BUFS, psum_bufs=PSUM_BUFS):
    ksub = IN // P + n_extra     # 42 K-slices of 128
    assert ksub % 2 == 0
    kpairs = ksub // 2
    msub = m_tile // M_SUB

    nc = bacc.Bacc(None, target_bir_lowering=False)
    xt = nc.declare_dram_parameter("xt", [P, ksub, TOK], mybir.dt.float8e4, False)
    wt = nc.declare_dram_parameter("wt", [P, NG, ksub, P], mybir.dt.float8e4, False)
    scale = nc.declare_dram_parameter("scale", [P, NG], mybir.dt.float32, False)
    bias = nc.declare_dram_parameter("bias", [P, NG], mybir.dt.float32, False)
    cr = nc.declare_dram_parameter("cr", [TOK], mybir.dt.float32, False)
    out = nc.declare_dram_parameter("out", [O_PAD, TOK], mybir.dt.float32, True)
    out_re = out.rearrange("(g p) t -> g p t", p=P)

    DR = mybir.MatmulPerfMode.DoubleRow
    ADD = mybir.AluOpType.add
    MULT = mybir.AluOpType.mult

    with tile.TileContext(nc) as tc:
        with (
            tc.tile_pool(name="const", bufs=1) as cpool,
            tc.tile_pool(name="xp", bufs=x_bufs) as xpool,
            tc.tile_pool(name="op", bufs=4) as opool,
            tc.tile_pool(name="ps", bufs=psum_bufs, space="PSUM") as pspool,
        ):
            # Startup: m-tile 0's x first, then the resident weights in
            # first-consumption order so matmul 0 starts after ~3.4MB of DMA.
            x0 = xpool.tile([P, ksub, m_tile], mybir.dt.float8e4, tag="x")
            nc.sync.dma_start(out=x0[:], in_=xt[:, :, 0:m_tile])
            w_tiles = []
            for g in range(NG):
                w_t = cpool.tile([P, ksub, P], mybir.dt.float8e4, tag=f"w{g}")
                nc.sync.dma_start(out=w_t[:], in_=wt[:, g])
                w_tiles.append(w_t)
            cr_sb = cpool.tile([P, TOK], mybir.dt.float32)
            nc.gpsimd.dma_start(out=cr_sb[:],
                                in_=cr[None, :].to_broadcast((P, TOK)))
            scale_sb = cpool.tile([P, NG], mybir.dt.float32)
            nc.gpsimd.dma_start(out=scale_sb[:], in_=scale[:, :])
            bias_sb = cpool.tile([P, NG], mybir.dt.float32)
            nc.gpsimd.dma_start(out=bias_sb[:], in_=bias[:, :])

            for mi in range(TOK // m_tile):
                if mi == 0:
                    x_sb = x0
                else:
                    x_sb = xpool.tile([P, ksub, m_tile], mybir.dt.float8e4,
                                      tag="x")
                    nc.sync.dma_start(
                        out=x_sb[:], in_=xt[:, :, mi * m_tile:(mi + 1) * m_tile])
                for ms in range(msub):
                    t0 = mi * m_tile + ms * M_SUB
                    for g in range(NG):
                        ps = pspool.tile([P, M_SUB], mybir.dt.float32, tag="ps")
                        for kp in range(kpairs):
                            nc.tensor.matmul(
                                ps[:],
                                lhsT=w_tiles[g][:, 2 * kp:2 * kp + 2, :],
                                rhs=x_sb[:, 2 * kp:2 * kp + 2,
                                         ms * M_SUB:(ms + 1) * M_SUB],
                                start=(kp == 0),
                                stop=(kp == kpairs - 1),
                                perf_mode=DR,
                            )
                        ob = opool.tile([P, M_SUB], mybir.dt.float32, tag="ob")
                        nc.vector.tensor_add(out=ob[:], in0=ps[:],
                                             in1=cr_sb[:, t0:t0 + M_SUB])
                        nc.vector.tensor_scalar(
                            out=ob[:], in0=ob[:],
                            scalar1=scale_sb[:, g:g + 1],
                            scalar2=bias_sb[:, g:g + 1],
                            op0=MULT, op1=ADD)
                        eng = nc.scalar if g % 2 else nc.gpsimd
                        eng.dma_start(out=out_re[g][:, t0:t0 + M_SUB], in_=ob[:])
    nc.compile()
    return nc


def _prep_inputs(x2d, w, scale, bias, n_extra=N_EXTRA):
    """Host-side quantization + swizzle. Returns per-core in_maps."""
    ksub = IN // P + n_extra
    kex = n_extra * P
    xq = x2d.astype(FP8)                                   # [TOK, IN]
    xlo = (x2d - xq.astype(np.float32))[:, :kex].astype(FP8)
    # K' x TOK, then swizzle to [P, ksub, TOK] with k = ks*P + p
    xt = np.concatenate([xq.T, xlo.T], axis=0)             # [K', TOK] fp8
    xt = np.ascontiguousarray(
        xt.reshape(ksub, P, TOK).transpose(1, 0, 2))       # [P, ksub, TOK]

    wq = ((w.astype(np.float32) - SHIFT) * ALPHA).astype(FP8)  # [OUT, IN]
    R = x2d.sum(axis=1, dtype=np.float64)
    cr = (ALPHA * SHIFT * R).astype(np.float32)            # [TOK]

    in_maps = []
    for c in range(N_CORES):
        sl = slice(c * O_CORE, (c + 1) * O_CORE)
        wc = np.zeros((O_PAD, IN), dtype=FP8)
        wc[:O_CORE] = wq[sl]
        wtc = np.concatenate([wc.T, wc.T[:kex]], axis=0)   # [K', O_PAD]
        wtc = np.ascontiguousarray(
            wtc.reshape(ksub, P, NG, P).transpose(1, 2, 0, 3))  # [P,NG,ksub,P]
        sc = np.zeros(O_PAD, dtype=np.float32)
        sc[:O_CORE] = scale[sl] / ALPHA
        bc = np.zeros(O_PAD, dtype=np.float32)
        bc[:O_CORE] = bias[sl]
        in_maps.append({
            "xt": xt,
            "wt": wtc,
            "scale": np.ascontiguousarray(sc.reshape(NG, P).T),
            "bias": np.ascontiguousarray(bc.reshape(NG, P).T),
            "cr": cr,
        })
    return in_maps


def _ensure_ntff_hook():
    """Register the axon NTFF profiling hook if the image's antenv lacks it."""
    import sys, types
    try:
        from antenv.axon_hooks import get_axon_ntff_profile_hook  # noqa: F401
        return
    except ImportError:
        pass
    try:
        import antenv
        from trn_agent_boot.trn_boot import _ntff_profile_via_ctypes
        mod = types.ModuleType("antenv.axon_hooks")
        _hook = [_ntff_profile_via_ctypes("/opt/axon/libaxon_pjrt.so")]
        mod.set_axon_ntff_profile_hook = lambda h: _hook.__setitem__(0, h)
        mod.get_axon_ntff_profile_hook = lambda: _hook[0]
        sys.modules["antenv.axon_hooks"] = mod
        antenv.axon_hooks = mod
    except Exception as e:  # profiling is best-effort; execution still works
        print(f"NTFF hook registration failed: {e}")


def run_hw(x2d, w, scale, bias, trace=False, **build_kwargs):
    """Run sharded on 8 cores; returns (full [TOK, OUT] f32 output, exec_ns)."""
    if trace:
        _ensure_ntff_hook()
    nc = build_nc(**build_kwargs)
    in_maps = _prep_inputs(x2d, w, scale, bias,
                           build_kwargs.get("n_extra", N_EXTRA))
    last_err = None
    for attempt in range(3):
        try:
            res = run_bass_kernel_spmd(nc, in_maps, core_ids=list(range(N_CORES)),
                                       trace=trace)
            parts = [res.results[c]["out"][:O_CORE] for c in range(N_CORES)]
            out = np.ascontiguousarray(np.concatenate(parts, axis=0).T)
            return out, res.exec_time_ns
        except Exception as e:  # transient NRT_EXEC_UNIT_UNRECOVERABLE etc.
            last_err = e
            print(f"run attempt {attempt} failed: {type(e).__name__}: {e}")
            try:
                import jax
                import jax.extend.backend as _jb
                jax.clear_caches()
                _jb.clear_backends()
            except Exception as e2:
                print(f"backend reset failed: {e2}")
            import time
            time.sleep(5)
    raise last_err


def kernel(**inputs):
    x = np.asarray(inputs["x"], dtype=np.float32)
    w = np.asarray(inputs["weight_int8"])
    scale = np.asarray(inputs["scale"], dtype=np.float32)
    bias = np.asarray(inputs["bias"], dtype=np.float32)
    out2d, _ = run_hw(x.reshape(TOK, IN), w, scale, bias, trace=False)
    return out2d.reshape(B, S, OUT)


# revision 6
# speedup vs baseline: 1.4964x; 1.0524x over previous
"""Trainium2 Bass kernel for CompressedLinear: out = x @ (w_int8 * scale).T + bias.

Sharding (Megatron column-parallel): weight/scale/bias are split along the
output dim across 8 NeuronCores, x is replicated, per-core outputs are
concatenated on the feature axis.

Strategy: fp8 e4m3 matmul in DoubleRow perf mode (2 K-slices contracted per
instruction -> 2x the fp16 PE rate).  Numerics to stay under the 2e-2 gate:
  - w codes [0,126] are mean-shifted (w-63) and scaled by ALPHA=1.0125 before
    the fp8 cast (the scan-optimized grid alignment: E[err^2] 0.88 -> 0.69);
    ALPHA and the shift are undone exactly at eviction (scale_eff = scale/ALPHA
    and a rank-1 token correction c_t = ALPHA*63*rowsum(x) added to PSUM,
    rowsums computed on host in f64).
  - x is cast to fp8; for the first N_EXTRA*128 of the 4096 K-dims the fp8
    residual (x - fp8(x)) is appended as extra K-slices against duplicated w
    slices, removing that fraction of the x-side quantization error.
    N_EXTRA=10 -> simulated rel err 1.892e-2 at 1.3125x the fp8 GEMM cost.

Layout: output channels live on PSUM partitions (128 per n-group, 11 groups =
1408 padded channels per core), tokens on the free dim.  scale/bias are then
per-partition operands of one DVE tensor_scalar, and the token correction is a
free-dim broadcast tile added with one DVE tensor_add.  Output is stored
channel-major [1408, TOK] per core (contiguous 1KB DMA lines) and transposed
on the host after the gather.
"""

import numpy as np
import ml_dtypes

import concourse.bass as bass
import concourse.mybir as mybir
import concourse.tile as tile
from concourse import bacc
from concourse.bass_utils import run_bass_kernel_spmd

B, S, IN, OUT = 4, 2048, 4096, 11008
N_CORES = 8
TOK = B * S
O_CORE = OUT // N_CORES          # 1376
P = 128
NG = 11                          # n-groups of 128 channels (1408 padded)
O_PAD = NG * P

N_EXTRA = 8                      # extra K-slices carrying x fp8 residuals
ALPHA = 1.0125                   # w quantizer grid scale (undone via scale_eff)
SHIFT = 63.0                     # w mean shift (undone via the rank-1 c term)
M_TILE = 512                     # tokens per x DMA tile
M_SUB = 256                      # tokens per PSUM tile (moving free = 512)
X_BUFS = 3
PSUM_BUFS = 6

FP8 = ml_dtypes.float8_e4m3


def build_nc(n_extra=N_EXTRA, m_tile=M_TILE, x_bufs=X_BUFS, psum_bufs=PSUM_BUFS):
    ksub = IN // P + n_extra     # 42 K-slices of 128
    assert ksub % 2 == 0
    kpairs = ksub // 2
    msub = m_tile // M_SUB

    nc = bacc.Bacc(None, target_bir_lowering=False)
    xt = nc.declare_dram_parameter("xt", [P, ksub, TOK], mybir.dt.float8e4, False)
    wt = nc.declare_dram_parameter("wt", [P, NG, ksub, P], mybir.dt.float8e4, False)
    scale = nc.declare_dram_parameter("scale", [P, NG], mybir.dt.float32, False)
    bias = nc.declare_dram_parameter("bias", [P, NG], mybir.dt.float32, False)
    cr = nc.declare_dram_parameter("cr", [TOK], mybir.dt.float32, False)
    out = nc.declare_dram_parameter("out", [O_PAD, TOK], mybir.dt.float32, True)
    out_re = out.rearrange("(g p) t -> g p t", p=P)

    DR = mybir.MatmulPerfMode.DoubleRow
    ADD = mybir.AluOpType.add
    MULT = mybir.AluOpType.mult

    with tile.TileContext(nc) as tc:
        with (
            tc.tile_pool(name="const", bufs=1) as cpool,
            tc.tile_pool(name="xp", bufs=x_bufs) as xpool,
            tc.tile_pool(name="op", bufs=4) as opool,
            tc.tile_pool(name="ps", bufs=psum_bufs, space="PSUM") as pspool,
        ):
            # Startup: interleave k-chunks of w-group 0 and m-tile 0's x in
            # first-consumption order (w on the vector ring, x on sync, so
            # both streams enqueue in parallel) -> the first PSUM group's
            # dependencies amount to ~300KB instead of 3.4MB of DMA.
            KG = 8
            kchunks = [(kc, min(KG, ksub - kc)) for kc in range(0, ksub, KG)]
            x0 = xpool.tile([P, ksub, m_tile], mybir.dt.float8e4, tag="x")
            w_tiles = [cpool.tile([P, ksub, P], mybir.dt.float8e4,
                                  name=f"w{g}", tag=f"w{g}") for g in range(NG)]
            for kc, kn in kchunks:
                nc.scalar.dma_start(out=w_tiles[0][:, kc:kc + kn, :],
                                    in_=wt[:, 0, kc:kc + kn, :])
                for ms in range(msub):
                    nc.sync.dma_start(
                        out=x0[:, kc:kc + kn, ms * M_SUB:(ms + 1) * M_SUB],
                        in_=xt[:, kc:kc + kn, ms * M_SUB:(ms + 1) * M_SUB])
            for g in range(1, NG):
                nc.scalar.dma_start(out=w_tiles[g][:], in_=wt[:, g])
            # scale/bias instantly, then the token-correction broadcast in
            # chunks so the first evictions don't wait on the whole 4MB.
            scale_sb = cpool.tile([P, NG], mybir.dt.float32)
            nc.gpsimd.dma_start(out=scale_sb[:], in_=scale[:, :])
            bias_sb = cpool.tile([P, NG], mybir.dt.float32)
            nc.gpsimd.dma_start(out=bias_sb[:], in_=bias[:, :])
            cr_sb = cpool.tile([P, TOK], mybir.dt.float32)
            for c0, c1 in [(0, 512), (512, 2048), (2048, 4096), (4096, 8192)]:
                nc.gpsimd.dma_start(
                    out=cr_sb[:, c0:c1],
                    in_=cr[None, c0:c1].to_broadcast((P, c1 - c0)))

            for mi in range(TOK // m_tile):
                if mi == 0:
                    x_sb = x0
                else:
                    x_sb = xpool.tile([P, ksub, m_tile], mybir.dt.float8e4,
                                      tag="x")
                    nc.sync.dma_start(
                        out=x_sb[:], in_=xt[:, :, mi * m_tile:(mi + 1) * m_tile])
                for ms in range(msub):
                    t0 = mi * m_tile + ms * M_SUB
                    for g in range(NG):
                        ps = pspool.tile([P, M_SUB], mybir.dt.float32, tag="ps")
                        for kp in range(kpairs):
                            nc.tensor.matmul(
                                ps[:],
                                lhsT=w_tiles[g][:, 2 * kp:2 * kp + 2, :],
                                rhs=x_sb[:, 2 * kp:2 * kp + 2,
                                         ms * M_SUB:(ms + 1) * M_SUB],
                                start=(kp == 0),
                                stop=(kp == kpairs - 1),
                                perf_mode=DR,
                            )
                        ob = opool.tile([P, M_SUB], mybir.dt.float32, tag="ob")
                        nc.vector.tensor_add(out=ob[:], in0=ps[:],
                                             in1=cr_sb[:, t0:t0 + M_SUB])
                        nc.vector.tensor_scalar(
                            out=ob[:], in0=ob[:],
                            scalar1=scale_sb[:, g:g + 1],
                            scalar2=bias_sb[:, g:g + 1],
                            op0=MULT, op1=ADD)
                        eng = nc.scalar if g % 2 else nc.gpsimd
                        eng.dma_start(out=out_re[g][:, t0:t0 + M_SUB], in_=ob[:])
    nc.compile()
    return nc


def _prep_inputs(x2d, w, scale, bias, n_extra=N_EXTRA):
    """Host-side quantization + swizzle. Returns per-core in_maps."""
    ksub = IN // P + n_extra
    kex = n_extra * P
    xq = x2d.astype(FP8)                                   # [TOK, IN]
    xlo = (x2d - xq.astype(np.float32))[:, :kex].astype(FP8)
    # K' x TOK, then swizzle to [P, ksub, TOK] with k = ks*P + p
    xt = np.concatenate([xq.T, xlo.T], axis=0)             # [K', TOK] fp8
    xt = np.ascontiguousarray(
        xt.reshape(ksub, P, TOK).transpose(1, 0, 2))       # [P, ksub, TOK]

    wq = ((w.astype(np.float32) - SHIFT) * ALPHA).astype(FP8)  # [OUT, IN]
    R = x2d.sum(axis=1, dtype=np.float64)
    cr = (ALPHA * SHIFT * R).astype(np.float32)            # [TOK]

    in_maps = []
    for c in range(N_CORES):
        sl = slice(c * O_CORE, (c + 1) * O_CORE)
        wc = np.zeros((O_PAD, IN), dtype=FP8)
        wc[:O_CORE] = wq[sl]
        wtc = np.concatenate([wc.T, wc.T[:kex]], axis=0)   # [K', O_PAD]
        wtc = np.ascontiguousarray(
            wtc.reshape(ksub, P, NG, P).transpose(1, 2, 0, 3))  # [P,NG,ksub,P]
        sc = np.zeros(O_PAD, dtype=np.float32)
        sc[:O_CORE] = scale[sl] / ALPHA
        bc = np.zeros(O_PAD, dtype=np.float32)
        bc[:O_CORE] = bias[sl]
        in_maps.append({
            "xt": xt,
            "wt": wtc,
            "scale": np.ascontiguousarray(sc.reshape(NG, P).T),
            "bias": np.ascontiguousarray(bc.reshape(NG, P).T),
            "cr": cr,
        })
    return in_maps


def _ensure_ntff_hook():
    """Register the axon NTFF profiling hook if the image's antenv lacks it."""
    import sys, types
    try:
        from antenv.axon_hooks import get_axon_ntff_profile_hook  # noqa: F401
        return
    except ImportError:
        pass
    try:
        import antenv
        from trn_agent_boot.trn_boot import _ntff_profile_via_ctypes
        mod = types.ModuleType("antenv.axon_hooks")
        _hook = [_ntff_profile_via_ctypes("/opt/axon/libaxon_pjrt.so")]
        mod.set_axon_ntff_profile_hook = lambda h: _hook.__setitem__(0, h)
        mod.get_axon_ntff_profile_hook = lambda: _hook[0]
        sys.modules["antenv.axon_hooks"] = mod
        antenv.axon_hooks = mod
    except Exception as e:  # profiling is best-effort; execution still works
        print(f"NTFF hook registration failed: {e}")


def run_hw(x2d, w, scale, bias, trace=False, **build_kwargs):
    """Run sharded on 8 cores; returns (full [TOK, OUT] f32 output, exec_ns)."""
    if trace:
        _ensure_ntff_hook()
    nc = build_nc(**build_kwargs)
    in_maps = _prep_inputs(x2d, w, scale, bias,
                           build_kwargs.get("n_extra", N_EXTRA))
    last_err = None
    for attempt in range(3):
        try:
            res = run_bass_kernel_spmd(nc, in_maps, core_ids=list(range(N_CORES)),
                                       trace=trace)
            parts = [res.results[c]["out"][:O_CORE] for c in range(N_CORES)]
            out = np.ascontiguousarray(np.concatenate(parts, axis=0).T)
            return out, res.exec_time_ns
        except Exception as e:  # transient NRT_EXEC_UNIT_UNRECOVERABLE etc.
            last_err = e
            print(f"run attempt {attempt} failed: {type(e).__name__}: {e}")
            try:
                import jax
                import jax.extend.backend as _jb
                jax.clear_caches()
                _jb.clear_backends()
            except Exception as e2:
                print(f"backend reset failed: {e2}")
            import time
            time.sleep(5)
    raise last_err


def kernel(**inputs):
    x = np.asarray(inputs["x"], dtype=np.float32)
    w = np.asarray(inputs["weight_int8"])
    scale = np.asarray(inputs["scale"], dtype=np.float32)
    bias = np.asarray(inputs["bias"], dtype=np.float32)
    out2d, _ = run_hw(x.reshape(TOK, IN), w, scale, bias, trace=False)
    return out2d.reshape(B, S, OUT)
